# revision 13
# baseline (speedup 1.0000x reference)
"""Multi-head attention forward for nn_AttentionStoreActivationPrune.

The reference's straight-through pattern ``sg(dense) + prune - sg(prune)``
is numerically ``dense`` in the forward pass, so every top-k masking branch
cancels and the output equals a plain multi-head attention forward.

Sharding: data-parallel over batch - 8 batch elements, one per NeuronCore.

Numerics strategy (cost model: matmul time = out_free_rows * cycles, with
fp8 DoubleRow = 0.5 cycles/row vs 1.0 for bf16/f32r):
  - QKV projections run as THREE error-compensated fp8e4m3 DoubleRow terms:
      X@W ~= X8@W8 + (X8/16)@dW8 + (dX8/16)@W8s
    with W prescaled by 32 (keeps fp8 normals; 56% of raw W entries would be
    subnormal), dW8 = f8(16*(32W - f8(32W))), W8s = f8(2W), dX8 = f8(16*(X -
    f8(X))).  Each term contracts 256/instruction (ko-paired DoubleRow), so a
    768-deep projection costs 4.5 cyc/col vs 6.0 for bf16, at ~bf16 accuracy.
  - Scores use the zero-slot DoubleRow trick: lhsT = K8 pairs with slot1
    zeroed, rhs = Q8 pairs with slot1 zero -> 0.5 cyc/row at contraction 64
    (the wasted slot is multiplied by zero).  Q/K evict to fp8 (the only
    lossy eviction; ~1.3% of the gate on its own, gate is 2e-2).
  - exp on the scalar engine writes bf16 E; softmax denominator rides along
    as a 65th V column (value 4.0, folding part of the 1/32 prescale).
  - ctx is sequence-major ([s_q, d] accumulation, 65-row moving dim) so the
    denominator is per-(partition,block) and normalization fuses into the
    PSUM eviction via a broadcast reciprocal multiply.  bf16.
  - ctx is PE-transposed (identity matmul) to feature-major for the bf16
    output projection; the residual 1/8 scale folds into the final eviction.

All intermediate evictions except Q/K are bf16 (fp8 anywhere else blows the
2e-2 gate - measured per-site).  Weights DMA as fp8 triples, Wo as bf16.

Biases are structurally zero in this problem (setup_inputs fills zeros);
kernel() checks and falls back to a with-bias program built on demand.
"""

import numpy as np
import ml_dtypes

S, H, NH, HD, KO = 577, 768, 12, 64, 6
B = 8
SQP = 579           # padded s_q: 3 * 193 (DoubleRow moving chunks)
SKP = 640           # padded s_k / x8 free size: 5 * 128
NQ8 = 193           # score / qk-projection moving chunk
SCH = [(0, 128), (128, 128), (256, 128), (384, 128), (512, 65)]
EXP_SCALE = 1.0 / 8192.0   # (1/8) / (32*32) : exp(scores/8) with 32x q,k

_CACHE = {}

F8 = ml_dtypes.float8_e4m3
BF16 = ml_dtypes.bfloat16


def _build_nc(zero_bias):
    import concourse.mybir as mybir
    import concourse.tile as tile
    from concourse import bacc

    f32 = mybir.dt.float32
    f8 = mybir.dt.float8e4
    bf16 = mybir.dt.bfloat16
    u8 = mybir.dt.uint8
    ADD = mybir.AluOpType.add
    MUL = mybir.AluOpType.mult
    EXP = mybir.ActivationFunctionType.Exp
    DR = mybir.MatmulPerfMode.DoubleRow

    nc = bacc.Bacc("TRN2", target_bir_lowering=False, debug=False)

    x8_d = nc.dram_tensor("x8", [128, KO, SKP], f8, kind="ExternalInput")
    x8s_d = nc.dram_tensor("x8s", [128, KO, SKP], f8, kind="ExternalInput")
    dx8_d = nc.dram_tensor("dx8", [128, KO, SKP], f8, kind="ExternalInput")
    wt_d = {}
    for nm in ("wq", "wk", "wv"):
        for pre in ("", "d", "s"):
            wt_d[pre + nm] = nc.dram_tensor(
                pre + nm, [128, KO, H], f8, kind="ExternalInput")
    wo_d = nc.dram_tensor("wo", [128, KO, H], bf16, kind="ExternalInput")
    id_d = nc.dram_tensor("ident", [128, 128], bf16, kind="ExternalInput")
    if not zero_bias:
        # biases pre-scaled on host: bq32/bk32 = 32*b (f32 cols), bv32 row =
        # 32*bv (bf16), bo8 row = 8*bo (bf16); ones row for rank-1 matmuls
        bq_d = nc.dram_tensor("bq32", [H], f32, kind="ExternalInput")
        bk_d = nc.dram_tensor("bk32", [H], f32, kind="ExternalInput")
        bv_d = nc.dram_tensor("bv32", [1, H], bf16, kind="ExternalInput")
        bo_d = nc.dram_tensor("bo8", [1, H], bf16, kind="ExternalInput")
        ones_d = nc.dram_tensor("ones", [1, 128], bf16, kind="ExternalInput")
    out_d = nc.dram_tensor("out", [S, H], bf16, kind="ExternalOutput")

    with tile.TileContext(nc) as tc:
        with tc.tile_pool(name="consts", bufs=1) as consts, \
             tc.tile_pool(name="wts", bufs=1) as wts, \
             tc.tile_pool(name="bigs", bufs=1) as bigs, \
             tc.tile_pool(name="epool", bufs=3) as epool, \
             tc.tile_pool(name="mid", bufs=4) as mid, \
             tc.tile_pool(name="outs", bufs=3) as outsp:

            ident = consts.tile([128, 128], bf16, tag="ident")
            nc.scalar.dma_start(out=ident, in_=id_d[:])
            warm = consts.tile([128, 2, 256], f8, tag="warm")
            nc.vector.memset(warm[:, :, :].bitcast(f32), 0.0)
            if not zero_bias:
                ones = consts.tile([1, 128], bf16, tag="ones")
                nc.scalar.dma_start(out=ones, in_=ones_d[:])
                bq_t = consts.tile([128, KO], f32, tag="bq")
                nc.scalar.dma_start(
                    out=bq_t, in_=bq_d.rearrange("(ko ki) -> ki ko", ki=128))
                bk_t = consts.tile([128, KO], f32, tag="bk")
                nc.scalar.dma_start(
                    out=bk_t, in_=bk_d.rearrange("(ko ki) -> ki ko", ki=128))
                bv_t = consts.tile([1, H], bf16, tag="bv")
                nc.scalar.dma_start(out=bv_t, in_=bv_d[:])
                bo_t = consts.tile([1, H], bf16, tag="bo")
                nc.scalar.dma_start(out=bo_t, in_=bo_d[:])

            # ---- big activation tiles ----
            X8 = bigs.tile([128, KO, SKP], f8, tag="X8")
            X8S = bigs.tile([128, KO, SKP], f8, tag="X8S")
            DX8 = bigs.tile([128, KO, SKP], f8, tag="DX8")
            # Q/K fp8 pair tiles: [:, 0, :] = value, [:, 1, :] = zeros
            QT8 = [bigs.tile([128, 2, SKP], f8, tag=f"QT{i}", name=f"QT{i}")
                   for i in range(KO)]
            KT8 = [bigs.tile([128, 2, SKP], f8, tag=f"KT{i}", name=f"KT{i}")
                   for i in range(KO)]
            # zero the pair tiles (slot1 must be 0 for the zero-slot trick;
            # K slot0 cols >= S must be 0 so padded scores rows exp to finite)
            for t in QT8 + KT8:
                nc.gpsimd.memset(t[:, :, :].bitcast(f32), 0.0)
            Vaug = [bigs.tile([128, NH, HD + 1], bf16, tag=f"vaug{i}",
                              name=f"vaug{i}")
                    for i in range(len(SCH))]
            for sc, (s0, sz) in enumerate(SCH):
                # denominator ride-along column; 4.0 folds part of the 32x
                # V prescale so ctx evicts at 8x the normalized value.
                # Pad rows (s_k >= 577, where E = exp(0) = 1) must carry 0 so
                # they don't inflate the denominator.
                if sz < 128:
                    # partition base must be 0/32/64/96: zero [64:128] first,
                    # then the 4.0 write below restores row 64 (s_k=576)
                    nc.vector.memset(Vaug[sc][64:128, :, HD:HD + 1], 0.0)
                nc.vector.memset(Vaug[sc][0:sz, :, HD:HD + 1], 4.0)
            CTXN = bigs.tile([128, 5, NH, HD], bf16, tag="CTXN")
            CTXT = bigs.tile([128, KO, SKP], bf16, tag="CTXT")

            w_t = {nm: wts.tile([128, KO, H], f8, tag=nm, name=nm)
                   for nm in wt_d}
            wo_t = wts.tile([128, KO, H], bf16, tag="wo")

            # ---- input DMAs in consumption order ----
            for k0 in range(0, KO, 2):
                nc.sync.dma_start(out=X8[:, k0:k0 + 2, :],
                                  in_=x8_d[:, k0:k0 + 2, :])
            for nm in ("wq", "wk"):
                for k0 in range(0, KO, 2):
                    nc.sync.dma_start(out=w_t[nm][:, k0:k0 + 2, :],
                                      in_=wt_d[nm][:, k0:k0 + 2, :])
            for k0 in range(0, KO, 2):
                nc.sync.dma_start(out=X8S[:, k0:k0 + 2, :],
                                  in_=x8s_d[:, k0:k0 + 2, :])
            for nm in ("dwq", "dwk"):
                for k0 in range(0, KO, 2):
                    nc.sync.dma_start(out=w_t[nm][:, k0:k0 + 2, :],
                                      in_=wt_d[nm][:, k0:k0 + 2, :])
            for k0 in range(0, KO, 2):
                nc.sync.dma_start(out=DX8[:, k0:k0 + 2, :],
                                  in_=dx8_d[:, k0:k0 + 2, :])
            for nm in ("swq", "swk", "wv", "dwv", "swv"):
                for k0 in range(0, KO, 2):
                    nc.sync.dma_start(out=w_t[nm][:, k0:k0 + 2, :],
                                      in_=wt_d[nm][:, k0:k0 + 2, :])
            for k0 in range(0, KO, 2):
                nc.sync.dma_start(out=wo_t[:, k0:k0 + 2, :],
                                  in_=wo_d[:, k0:k0 + 2, :])

            TERMS = [("X", ""), ("XS", "d"), ("DX", "s")]

            def xterm(t):
                return {"X": X8, "XS": X8S, "DX": DX8}[t]

            with tc.tile_pool(name="pproj", bufs=1, space="PSUM") as pproj:

                for wi in range(40):
                    pw = pproj.tile([128, 3, 512], f32, tag="pq",
                                    name=f"warm{wi}")
                    nc.tensor.matmul(pw[:, 0, 0:256], warm[:, :, 0:128],
                                     warm[:, :, :], start=True, stop=True,
                                     perf_mode=DR)

                def project_qk(koh):
                    c0 = koh * 128
                    for dst, wname, bias in ((QT8[koh], "wq", "bq"),
                                             (KT8[koh], "wk", "bk")):
                        pq = pproj.tile([128, 3, 512], f32, tag="pq",
                                        name=f"pq_{wname}_{koh}")
                        for ti, (xt, wpre) in enumerate(TERMS):
                            wsrc = w_t[wpre + wname]
                            for ko in range(3):
                                for qc in range(3):
                                    nc.tensor.matmul(
                                        pq[:, qc, 0:NQ8],
                                        wsrc[:, 2 * ko:2 * ko + 2,
                                             c0:c0 + 128],
                                        xterm(xt)[:, 2 * ko:2 * ko + 2,
                                                  qc * NQ8:(qc + 1) * NQ8],
                                        start=(ti == 0 and ko == 0),
                                        stop=(xt == "DX" and ko == 2
                                              and qc == 2),
                                        perf_mode=DR,
                                    )
                        dst_v = dst[:, 0, 0:SQP].rearrange(
                            "p (c b) -> p c b", b=NQ8)
                        if zero_bias:
                            nc.vector.tensor_copy(
                                out=dst_v, in_=pq[:, :, 0:NQ8])
                        else:
                            bias_t = bq_t if bias == "bq" else bk_t
                            nc.vector.tensor_scalar(
                                dst_v, pq[:, :, 0:NQ8],
                                bias_t[:, koh:koh + 1], None, ADD)

                def project_v():
                    for sc, (s0, sz) in enumerate(SCH):
                        pv = pproj.tile([128, 3, 512], f32, tag="pq",
                                        name=f"pv{sc}")
                        for ti, (xt, wpre) in enumerate(TERMS):
                            wsrc = w_t[wpre + "wv"]
                            for ko in range(3):
                                for vc in range(3):
                                    nc.tensor.matmul(
                                        pv[:, vc, 0:256],
                                        xterm(xt)[:, 2 * ko:2 * ko + 2,
                                                  s0:s0 + 128],
                                        wsrc[:, 2 * ko:2 * ko + 2,
                                             vc * 256:(vc + 1) * 256],
                                        start=(ti == 0 and ko == 0),
                                        stop=(xt == "DX" and ko == 2
                                              and vc == 2 and zero_bias),
                                        perf_mode=DR,
                                    )
                        if not zero_bias:
                            for vc in range(3):
                                nc.tensor.matmul(
                                    pv[:, vc, 0:256],
                                    ones[0:1, 0:128],
                                    bv_t[0:1, vc * 256:(vc + 1) * 256],
                                    start=False, stop=(vc == 2),
                                )
                        nc.vector.tensor_copy(
                            out=Vaug[sc][:, :, 0:HD].rearrange(
                                "p (c h) d -> p c h d", c=3),
                            in_=pv[:, :, 0:256].rearrange(
                                "p c (h d) -> p c h d", d=HD))

                project_qk(0)
                project_v()
                project_qk(1)

                with tc.tile_pool(name="pscore", bufs=2, space="PSUM") \
                        as pscore, \
                     tc.tile_pool(name="pctx", bufs=1, space="PSUM") as pctx:

                    def attend(h):
                        koh, kb = h // 2, (h % 2) * HD
                        pcs = pctx.tile([128, 5, HD + 1], f32, tag="pc",
                                        name=f"pc{h}")
                        # zeroing matmul covering the whole accumulator bank
                        # (single start instruction; all real ctx matmuls
                        # accumulate with start=False)
                        nc.tensor.matmul(
                            pcs[:, :, :].rearrange("p a b -> p (a b)"),
                            warm[:, 0, 0:128], X8[:, 0, 0:5 * (HD + 1)],
                            start=True, stop=False,
                        )
                        for sc in range(len(SCH)):
                            ps = pscore.tile([128, 3, 256], f32, tag="ps",
                                             name=f"ps{h}_{sc}")
                            for qc in range(3):
                                nc.tensor.matmul(
                                    ps[:, qc, 0:NQ8],
                                    KT8[koh][kb:kb + HD, :,
                                             sc * 128:sc * 128 + 128],
                                    QT8[koh][kb:kb + HD, :,
                                             qc * NQ8:(qc + 1) * NQ8],
                                    start=True, stop=True, perf_mode=DR,
                                )
                            E = epool.tile([128, SKP], bf16, tag="e",
                                           name=f"e{h}_{sc}")
                            nc.scalar.activation(
                                out=E[:, 0:SQP].rearrange(
                                    "p (c b) -> p c b", b=NQ8),
                                in_=ps[:, :, 0:NQ8],
                                func=EXP, scale=EXP_SCALE,
                            )
                            for qb in range(5):
                                nc.tensor.matmul(
                                    pcs[:, qb, :],
                                    E[:, qb * 128:qb * 128 + 128],
                                    Vaug[sc][:, h, :],
                                    start=False,
                                    stop=(sc == len(SCH) - 1 and qb == 4),
                                )
                        rec = mid.tile([128, 5], f32, tag="rec",
                                       name=f"rec{h}")
                        nc.vector.reciprocal(out=rec, in_=pcs[:, :, HD:HD + 1])
                        nc.vector.tensor_tensor(
                            out=CTXN[:, :, h, :],
                            in0=pcs[:, :, 0:HD],
                            in1=rec[:, :, None].broadcast_to([128, 5, HD]),
                            op=MUL,
                        )

                    for koh in range(KO):
                        if koh + 2 < KO:
                            project_qk(koh + 2)
                        attend(2 * koh)
                        attend(2 * koh + 1)

            with tc.tile_pool(name="pout", bufs=2, space="PSUM") as pout, \
                 tc.tile_pool(name="ptr", bufs=2, space="PSUM") as ptrp:
                # ---- transpose ctx to feature-major, then out projection ----
                for qb, (s0, sz) in enumerate(SCH):
                    for fg in range(2):
                        ptr = ptrp.tile([128, 3, 128], bf16, tag="ptr",
                                        name=f"ptr{qb}_{fg}")
                        for fi in range(3):
                            fo = fg * 3 + fi
                            nc.tensor.transpose(
                                ptr[:, fi, :],
                                CTXN[:, qb, 2 * fo:2 * fo + 2, :], ident)
                        nc.vector.tensor_copy(
                            out=CTXT[:, fg * 3:fg * 3 + 3, s0:s0 + 128],
                            in_=ptr)
                for sc, (s0, sz) in enumerate(SCH):
                    po = pout.tile([128, H], f32, tag="po", name=f"po{sc}")
                    for ko in range(KO):
                        for oc, (n0, nn) in enumerate(((0, 512), (512, 256))):
                            nc.tensor.matmul(
                                po[:, n0:n0 + nn],
                                CTXT[:, ko, s0:s0 + 128],
                                wo_t[:, ko, n0:n0 + nn],
                                start=(ko == 0),
                                stop=(ko == KO - 1 and oc == 1 and zero_bias),
                            )
                    if not zero_bias:
                        for oc, (n0, nn) in enumerate(((0, 512), (512, 256))):
                            nc.tensor.matmul(
                                po[:, n0:n0 + nn],
                                ones[0:1, 0:128],
                                bo_t[0:1, n0:n0 + nn],
                                start=False, stop=(oc == 1),
                            )
                    osb = outsp.tile([128, H], bf16, tag="osb")
                    nc.vector.tensor_scalar_mul(osb, po, 0.125)
                    deng = nc.sync if sc % 2 == 0 else nc.scalar
                    deng.dma_start(out=out_d[s0:s0 + sz, :],
                                   in_=osb[0:sz, :])

    nc.finalize()
    return nc


def _prep(hidden_states, Wq, Wk, Wv, Wo):
    """Host-side prep: fp8 triples for X and the QKV weights, bf16 Wo."""
    f8 = lambda a: np.asarray(a, F8)
    hs = np.ascontiguousarray(hidden_states, np.float32)
    xt = np.zeros((B, 128, KO, SKP), np.float32)
    xt[:, :, :, :S] = hs.transpose(0, 2, 1).reshape(B, KO, 128, S) \
        .transpose(0, 2, 1, 3)
    x8 = f8(xt)
    x8s = f8(xt / 16.0)
    dx8 = f8(16.0 * (xt - x8.astype(np.float32)))

    def wtrip(W):
        Wp = 32.0 * np.ascontiguousarray(W, np.float32)
        Wr = Wp.reshape(KO, 128, H).transpose(1, 0, 2)  # [ki, ko, o]
        w8 = f8(Wr)
        dw8 = f8(16.0 * (Wr - w8.astype(np.float32)))
        w8s = f8(Wr / 16.0)
        return w8, dw8, w8s

    wq8, dwq8, swq8 = wtrip(Wq)
    wk8, dwk8, swk8 = wtrip(Wk)
    wv8, dwv8, swv8 = wtrip(Wv)
    wo16 = np.asarray(
        np.ascontiguousarray(Wo, np.float32).reshape(KO, 128, H)
        .transpose(1, 0, 2), BF16)
    common = {
        "wq": wq8, "dwq": dwq8, "swq": swq8,
        "wk": wk8, "dwk": dwk8, "swk": swk8,
        "wv": wv8, "dwv": dwv8, "swv": swv8,
        "wo": wo16, "ident": np.eye(128, dtype=BF16),
    }
    return common, x8, x8s, dx8


def kernel(hidden_states, Wq, bq, Wk, bk, Wv, bv, Wo, bo):
    from concourse.bass_utils import run_bass_kernel_spmd

    zero_bias = not (np.any(bq) or np.any(bk) or np.any(bv) or np.any(bo))
    key = ("nc", zero_bias)
    if key not in _CACHE:
        _CACHE[key] = _build_nc(zero_bias)
    nc = _CACHE[key]

    common, x8, x8s, dx8 = _prep(hidden_states, Wq, Wk, Wv, Wo)
    if not zero_bias:
        common.update({
            "bq32": np.ascontiguousarray(32.0 * bq, np.float32),
            "bk32": np.ascontiguousarray(32.0 * bk, np.float32),
            "bv32": np.asarray(32.0 * bv, BF16).reshape(1, H),
            "bo8": np.asarray(8.0 * bo, BF16).reshape(1, H),
            "ones": np.ones((1, 128), BF16),
        })
    in_maps = [dict(common, x8=x8[b], x8s=x8s[b], dx8=dx8[b])
               for b in range(B)]

    res = run_bass_kernel_spmd(nc, in_maps, core_ids=list(range(B)))
    out = np.stack([np.asarray(r["out"]).astype(np.float32)
                    for r in res.results], axis=0)
    return out


if __name__ == "__main__":
    rng = np.random.default_rng(0)
    inputs = {
        "hidden_states": rng.standard_normal((B, S, H)).astype(np.float32),
        "Wq": (rng.standard_normal((H, H)) * 0.02).astype(np.float32),
        "bq": np.zeros(H, np.float32),
        "Wk": (rng.standard_normal((H, H)) * 0.02).astype(np.float32),
        "bk": np.zeros(H, np.float32),
        "Wv": (rng.standard_normal((H, H)) * 0.02).astype(np.float32),
        "bv": np.zeros(H, np.float32),
        "Wo": (rng.standard_normal((H, H)) * 0.02).astype(np.float32),
        "bo": np.zeros(H, np.float32),
    }
    got = kernel(**inputs)
    print("kernel output:", got.shape, got.dtype)


# revision 17
# speedup vs baseline: 1.1082x; 1.1082x over previous
"""Multi-head attention forward for nn_AttentionStoreActivationPrune.

The reference's straight-through pattern ``sg(dense) + prune - sg(prune)``
is numerically ``dense`` in the forward pass, so every top-k masking branch
cancels and the output equals a plain multi-head attention forward.

Sharding: data-parallel over batch - 8 batch elements, one per NeuronCore.

Numerics strategy (cost model: matmul time = out_free_rows * cycles, with
fp8 DoubleRow = 0.5 cycles/row vs 1.0 for bf16/f32r):
  - QKV projections run as THREE error-compensated fp8e4m3 DoubleRow terms:
      X@W ~= X8@W8 + (X8/16)@dW8 + (dX8/16)@W8s
    with W prescaled by 32 (keeps fp8 normals; 56% of raw W entries would be
    subnormal), dW8 = f8(16*(32W - f8(32W))), W8s = f8(2W), dX8 = f8(16*(X -
    f8(X))).  Each term contracts 256/instruction (ko-paired DoubleRow), so a
    768-deep projection costs 4.5 cyc/col vs 6.0 for bf16, at ~bf16 accuracy.
  - Scores use the zero-slot DoubleRow trick: lhsT = K8 pairs with slot1
    zeroed, rhs = Q8 pairs with slot1 zero -> 0.5 cyc/row at contraction 64
    (the wasted slot is multiplied by zero).  Q/K evict to fp8 (the only
    lossy eviction; ~1.3% of the 2e-2 gate on its own).
  - exp on the scalar engine (the kernel's second bottleneck, ~35us) writes
    bf16 E in two-chunk batched calls; the softmax denominator rides along
    as a 65th V column (value 4.0, folding part of the 1/32 prescale).
  - ctx is sequence-major ([s_q, d] accumulation, 65-row moving dim) so the
    denominator is per-(partition,block) and normalization fuses into the
    PSUM eviction via a broadcast reciprocal multiply.  bf16.
  - ctx is PE-transposed (identity matmul) per head-pair to feature-major
    for the bf16 output projection; the 1/8 scale folds into the final
    eviction (on ACT, idle in the tail).

Scheduling: engines execute in program order, so each head's attention is
software-pipelined (scores of both chunk-pairs and PE filler - the next
head-pair's QK projection and the previous pair's ctx transposes - are
emitted BEFORE the exp-gated ctx matmuls).  A single 3-bank PSUM pool
serves projection and score tiles so projections overlap the ACT-bound
attention phase.  Inputs arrive as a few large packed DMAs (HWDGE issue
overhead is ~632ns each, serialized).

Biases are structurally zero in this problem (setup_inputs fills zeros);
kernel() checks and falls back to a with-bias program built on demand.
"""

import numpy as np
import ml_dtypes

S, H, NH, HD, KO = 577, 768, 12, 64, 6
B = 8
SQP = 579           # padded s_q: 3 * 193 (DoubleRow moving chunks)
SKP = 640           # padded s_k / x8 free size: 5 * 128
NQ8 = 193           # score / qk-projection moving chunk
SCH = [(0, 128), (128, 128), (256, 128), (384, 128), (512, 65)]
EXP_SCALE = 1.0 / 8192.0   # (1/8) / (32*32) : exp(scores/8) with 32x q,k

_CACHE = {}

F8 = ml_dtypes.float8_e4m3
BF16 = ml_dtypes.bfloat16


def _build_nc(zero_bias):
    import concourse.mybir as mybir
    import concourse.tile as tile
    from concourse import bacc

    f32 = mybir.dt.float32
    f8 = mybir.dt.float8e4
    bf16 = mybir.dt.bfloat16
    ADD = mybir.AluOpType.add
    MUL = mybir.AluOpType.mult
    EXP = mybir.ActivationFunctionType.Exp
    CPY = mybir.ActivationFunctionType.Copy
    DR = mybir.MatmulPerfMode.DoubleRow

    nc = bacc.Bacc("TRN2", target_bir_lowering=False, debug=False)

    x8_d = nc.dram_tensor("x8", [128, KO, SKP], f8, kind="ExternalInput")
    xres_d = nc.dram_tensor("xres", [128, KO, 2, SKP], f8,
                            kind="ExternalInput")
    # packed per-column-block QK weight triples: [128, ko, {q,k}, term, 128]
    wqk_d = [nc.dram_tensor(f"wqk{b}", [128, KO, 2, 3, 128], f8,
                            kind="ExternalInput") for b in range(KO)]
    wv8_d = nc.dram_tensor("wv8", [128, KO, H], f8, kind="ExternalInput")
    wvres_d = nc.dram_tensor("wvres", [128, KO, 2, H], f8,
                             kind="ExternalInput")
    wo_d = nc.dram_tensor("wo", [128, KO, H], bf16, kind="ExternalInput")
    id_d = nc.dram_tensor("ident", [128, 128], bf16, kind="ExternalInput")
    if not zero_bias:
        bq_d = nc.dram_tensor("bq32", [H], f32, kind="ExternalInput")
        bk_d = nc.dram_tensor("bk32", [H], f32, kind="ExternalInput")
        bv_d = nc.dram_tensor("bv32", [1, H], bf16, kind="ExternalInput")
        bo_d = nc.dram_tensor("bo8", [1, H], bf16, kind="ExternalInput")
        ones_d = nc.dram_tensor("ones", [1, 128], bf16, kind="ExternalInput")
    out_d = nc.dram_tensor("out", [S, H], bf16, kind="ExternalOutput")

    with tile.TileContext(nc) as tc:
        with tc.tile_pool(name="consts", bufs=1) as consts, \
             tc.tile_pool(name="wts", bufs=1) as wts, \
             tc.tile_pool(name="bigs", bufs=1) as bigs, \
             tc.tile_pool(name="epool", bufs=3) as epool, \
             tc.tile_pool(name="mid", bufs=4) as mid, \
             tc.tile_pool(name="outs", bufs=3) as outsp:

            ident = consts.tile([128, 128], bf16, tag="ident")
            nc.scalar.dma_start(out=ident, in_=id_d[:])
            warm = consts.tile([128, 2, 256], f8, tag="warm")
            nc.vector.memset(warm[:, :, :].bitcast(f32), 0.0)
            if not zero_bias:
                ones = consts.tile([1, 128], bf16, tag="ones")
                nc.scalar.dma_start(out=ones, in_=ones_d[:])
                bq_t = consts.tile([128, KO], f32, tag="bq")
                nc.scalar.dma_start(
                    out=bq_t, in_=bq_d.rearrange("(ko ki) -> ki ko", ki=128))
                bk_t = consts.tile([128, KO], f32, tag="bk")
                nc.scalar.dma_start(
                    out=bk_t, in_=bk_d.rearrange("(ko ki) -> ki ko", ki=128))
                bv_t = consts.tile([1, H], bf16, tag="bv")
                nc.scalar.dma_start(out=bv_t, in_=bv_d[:])
                bo_t = consts.tile([1, H], bf16, tag="bo")
                nc.scalar.dma_start(out=bo_t, in_=bo_d[:])

            # ---- big activation tiles ----
            X8 = bigs.tile([128, KO, SKP], f8, tag="X8")
            XRES = bigs.tile([128, KO, 2, SKP], f8, tag="XRES")
            QT8 = [bigs.tile([128, 2, SKP], f8, tag=f"QT{i}", name=f"QT{i}")
                   for i in range(KO)]
            KT8 = [bigs.tile([128, 2, SKP], f8, tag=f"KT{i}", name=f"KT{i}")
                   for i in range(KO)]
            # zero the pair tiles (slot1 must be 0 for the zero-slot trick;
            # K slot0 cols >= S must be 0 so padded score rows exp to finite)
            for t in QT8 + KT8:
                nc.gpsimd.memset(t[:, :, :].bitcast(f32), 0.0)
            Vaug = [bigs.tile([128, NH, HD + 1], bf16, tag=f"vaug{i}",
                              name=f"vaug{i}")
                    for i in range(len(SCH))]
            for sc, (s0, sz) in enumerate(SCH):
                # denominator ride-along column (4.0 folds part of the 32x V
                # prescale).  Pad rows (s_k >= 577, where E = exp(0) = 1)
                # must carry 0 so they don't inflate the denominator.
                if sz < 128:
                    nc.vector.memset(Vaug[sc][64:128, :, HD:HD + 1], 0.0)
                nc.vector.memset(Vaug[sc][0:sz, :, HD:HD + 1], 4.0)
            CTXN = bigs.tile([128, 5, NH, HD], bf16, tag="CTXN")
            CTXT = bigs.tile([128, KO, SKP], bf16, tag="CTXT")

            wqk_t = [wts.tile([128, KO, 2, 3, 128], f8, tag=f"wqk{b}",
                              name=f"wqk{b}") for b in range(KO)]
            wv8_t = wts.tile([128, KO, H], f8, tag="wv8")
            wvres_t = wts.tile([128, KO, 2, H], f8, tag="wvres")
            wo_t = wts.tile([128, KO, H], bf16, tag="wo")

            # ---- input DMAs, few and large, in consumption order ----
            nc.sync.dma_start(out=X8, in_=x8_d[:])
            nc.sync.dma_start(out=wv8_t, in_=wv8_d[:])
            nc.sync.dma_start(out=wqk_t[0], in_=wqk_d[0][:])
            nc.sync.dma_start(out=XRES, in_=xres_d[:])
            nc.sync.dma_start(out=wvres_t, in_=wvres_d[:])
            nc.sync.dma_start(out=wqk_t[1], in_=wqk_d[1][:])
            for b in range(2, KO):
                nc.sync.dma_start(out=wqk_t[b], in_=wqk_d[b][:])
            nc.sync.dma_start(out=wo_t, in_=wo_d[:])

            # X-side operand per compensation term
            def xop(t, ko, lo, hi):
                if t == 0:
                    return X8[:, 2 * ko:2 * ko + 2, lo:hi]
                return XRES[:, 2 * ko:2 * ko + 2, t - 1, lo:hi]

            pscore_cm = tc.tile_pool(name="pscore", bufs=2, space="PSUM")
            pproj_cm = tc.tile_pool(name="pproj", bufs=1, space="PSUM")
            pctx_cm = tc.tile_pool(name="pctx", bufs=1, space="PSUM")
            pscore = pscore_cm.__enter__()
            pproj = pproj_cm.__enter__()
            pctx = pctx_cm.__enter__()

            for wi in range(40):
                pw = pproj.tile([128, 512], f32, tag="pp",
                                name=f"warm{wi}")
                nc.tensor.matmul(pw[:, 0:256], warm[:, :, 0:128],
                                 warm[:, :, :], start=True, stop=True,
                                 perf_mode=DR)

            def qk_chunk(koh, iw, qc):
                """One projection moving-chunk: a 9-matmul 1-bank PSUM
                accumulation group plus its fp8 eviction."""
                pq = pproj.tile([128, 512], f32, tag="pp",
                                name=f"pq_{iw}_{koh}_{qc}")
                for it in range(3):
                    for ko in range(3):
                        nc.tensor.matmul(
                            pq[:, 0:NQ8],
                            wqk_t[koh][:, 2 * ko:2 * ko + 2, iw, it, :],
                            xop(it, ko, qc * NQ8, (qc + 1) * NQ8),
                            start=(it == 0 and ko == 0),
                            stop=(it == 2 and ko == 2),
                            perf_mode=DR,
                        )
                dst = (QT8, KT8)[iw][koh]
                dst_v = dst[:, 0, qc * NQ8:(qc + 1) * NQ8]
                if zero_bias:
                    nc.vector.tensor_copy(out=dst_v, in_=pq[:, 0:NQ8])
                else:
                    bias_t = bq_t if iw == 0 else bk_t
                    nc.vector.tensor_scalar(
                        dst_v, pq[:, 0:NQ8],
                        bias_t[:, koh:koh + 1], None, ADD)

            def project_qk(koh):
                for iw in range(2):
                    for qc in range(3):
                        qk_chunk(koh, iw, qc)

            def v_chunk(sc, vc):
                s0, sz = SCH[sc]
                pv = pproj.tile([128, 512], f32, tag="pp",
                                name=f"pv{sc}_{vc}")

                def vop(t, ko):
                    if t == 0:
                        return wv8_t[:, 2 * ko:2 * ko + 2, :]
                    return wvres_t[:, 2 * ko:2 * ko + 2, t - 1, :]

                for it in range(3):
                    for ko in range(3):
                        nc.tensor.matmul(
                            pv[:, 0:256],
                            xop(it, ko, s0, s0 + 128),
                            vop(it, ko)[..., vc * 256:(vc + 1) * 256],
                            start=(it == 0 and ko == 0),
                            stop=(it == 2 and ko == 2 and zero_bias),
                            perf_mode=DR,
                        )
                if not zero_bias:
                    nc.tensor.matmul(
                        pv[:, 0:256],
                        ones[0:1, 0:128],
                        bv_t[0:1, vc * 256:(vc + 1) * 256],
                        start=False, stop=True,
                    )
                nc.vector.tensor_copy(
                    out=Vaug[sc][:, 4 * vc:4 * vc + 4, 0:HD],
                    in_=pv[:, 0:256].rearrange("p (h d) -> p h d", d=HD))

            def project_v_block(sc):
                for vc in range(3):
                    v_chunk(sc, vc)

            def transpose_koh(koh):
                """Transpose heads 2koh,2koh+1 of CTXN into CTXT row-block
                koh: 5 PE transposes + one DVE eviction (pproj pool, bf16
                fits in the same 1-bank footprint)."""
                ptr_f = pproj.tile([128, 512], f32, tag="pp",
                                   name=f"ptr{koh}")
                ptr = ptr_f[:, 0:320].bitcast(bf16).rearrange(
                    "p (a b) -> p a b", b=128)
                for qb in range(5):
                    nc.tensor.transpose(
                        ptr[:, qb, :],
                        CTXN[:, qb, 2 * koh:2 * koh + 2, :], ident)
                nc.vector.tensor_copy(
                    out=CTXT[:, koh, :],
                    in_=ptr[:, :, :].rearrange("p a b -> p (a b)"))

            # score chunk-pairs: (sc0,sc1), (sc2,sc3), (sc4,)
            PAIRS = [(0, 1), (2, 3), (4,)]

            def emit_scores(h, pair_i):
                koh, kb = h // 2, (h % 2) * HD
                scs = PAIRS[pair_i]
                ps = pscore.tile([128, 2, 3, 256], f32, tag="ps",
                               name=f"ps{h}_{pair_i}")
                for pi, sc in enumerate(scs):
                    for qc in range(3):
                        nc.tensor.matmul(
                            ps[:, pi, qc, 0:NQ8],
                            KT8[koh][kb:kb + HD, :,
                                     sc * 128:sc * 128 + 128],
                            QT8[koh][kb:kb + HD, :,
                                     qc * NQ8:(qc + 1) * NQ8],
                            start=True, stop=True, perf_mode=DR,
                        )
                return ps

            def emit_exp(h, pair_i, ps):
                n = len(PAIRS[pair_i])
                E = epool.tile([128, 2, SKP], bf16, tag="e",
                               name=f"e{h}_{pair_i}")
                nc.scalar.activation(
                    out=E[:, 0:n, 0:SQP].rearrange(
                        "p a (c b) -> p a c b", b=NQ8),
                    in_=ps[:, 0:n, :, 0:NQ8],
                    func=EXP, scale=EXP_SCALE,
                )
                return E

            def emit_ctx(h, pair_i, E, pcs):
                for pi, sc in enumerate(PAIRS[pair_i]):
                    for qb in range(5):
                        nc.tensor.matmul(
                            pcs[:, qb, :],
                            E[:, pi, qb * 128:qb * 128 + 128],
                            Vaug[sc][:, h, :],
                            start=False,
                            stop=(sc == 4 and qb == 4),
                        )

            def ctx_dummy(h, pcs):
                # single start instruction zeroing the whole accumulator bank
                nc.tensor.matmul(
                    pcs[:, :, :].rearrange("p a b -> p (a b)"),
                    warm[:, 0, 0:128], X8[:, 0, 0:5 * (HD + 1)],
                    start=True, stop=False,
                )

            def finish_head(h, pcs):
                rec = mid.tile([128, 5], f32, tag="rec", name=f"rec{h}")
                nc.vector.reciprocal(out=rec, in_=pcs[:, :, HD:HD + 1])
                nc.vector.tensor_tensor(
                    out=CTXN[:, :, h, :],
                    in0=pcs[:, :, 0:HD],
                    in1=rec[:, :, None].broadcast_to([128, 5, HD]),
                    op=MUL,
                )

            def attend(h, fillers):
                """Software-pipelined head: scores/exp run ahead, PE filler
                work is emitted before each exp-gated ctx block."""
                ps0 = emit_scores(h, 0)
                E0 = emit_exp(h, 0, ps0)
                ps1 = emit_scores(h, 1)
                E1 = emit_exp(h, 1, ps1)
                pcs = pctx.tile([128, 5, HD + 1], f32, tag="pc",
                                name=f"pc{h}")
                ctx_dummy(h, pcs)
                for _ in range(2):
                    if fillers:
                        fillers.pop(0)()
                emit_ctx(h, 0, E0, pcs)
                ps2 = emit_scores(h, 2)
                E2 = emit_exp(h, 2, ps2)
                if fillers:
                    fillers.pop(0)()
                emit_ctx(h, 1, E1, pcs)
                emit_ctx(h, 2, E2, pcs)
                finish_head(h, pcs)

            # ---- phase A: first QK projection + V (term-major so the PE
            # starts on what has arrived) ----
            project_qk(0)
            for sc in range(len(SCH)):
                project_v_block(sc)
            project_qk(1)

            # ---- phase B: attention, QK projections and ctx transposes
            # interleaved as PE filler under the ACT-bound exp stream ----
            for koh in range(KO):
                fillers = []
                if koh + 2 < KO:
                    for iw in range(2):
                        for qc in range(3):
                            fillers.append(
                                lambda k=koh + 2, i=iw, q=qc:
                                qk_chunk(k, i, q))
                if koh > 0:
                    fillers.append(lambda k=koh - 1: transpose_koh(k))
                attend(2 * koh, fillers)
                attend(2 * koh + 1, fillers)
                for f in fillers:
                    f()
            transpose_koh(KO - 1)

            for p in (pctx_cm, pproj_cm, pscore_cm):
                p.__exit__(None, None, None)

            # ---- phase C: output projection (bf16), osb evict on ACT ----
            with tc.tile_pool(name="pout", bufs=2, space="PSUM") as pout:
                for sc, (s0, sz) in enumerate(SCH):
                    po = pout.tile([128, H], f32, tag="po", name=f"po{sc}")
                    for ko in range(KO):
                        for oc, (n0, nn) in enumerate(((0, 512), (512, 256))):
                            nc.tensor.matmul(
                                po[:, n0:n0 + nn],
                                CTXT[:, ko, s0:s0 + 128],
                                wo_t[:, ko, n0:n0 + nn],
                                start=(ko == 0),
                                stop=(ko == KO - 1 and oc == 1 and zero_bias),
                            )
                    if not zero_bias:
                        for oc, (n0, nn) in enumerate(((0, 512), (512, 256))):
                            nc.tensor.matmul(
                                po[:, n0:n0 + nn],
                                ones[0:1, 0:128],
                                bo_t[0:1, n0:n0 + nn],
                                start=False, stop=(oc == 1),
                            )
                    osb = outsp.tile([128, H], bf16, tag="osb")
                    nc.scalar.activation(out=osb, in_=po, func=CPY,
                                         scale=0.125)
                    deng = nc.sync if sc % 2 == 0 else nc.scalar
                    deng.dma_start(out=out_d[s0:s0 + sz, :],
                                   in_=osb[0:sz, :])

    nc.finalize()
    return nc


def _prep(hidden_states, Wq, Wk, Wv, Wo):
    """Host-side prep: fp8 triples for X and the QKV weights, bf16 Wo."""
    f8 = lambda a: np.asarray(a, F8)
    hs = np.ascontiguousarray(hidden_states, np.float32)
    xt = np.zeros((B, 128, KO, SKP), np.float32)
    xt[:, :, :, :S] = hs.transpose(0, 2, 1).reshape(B, KO, 128, S) \
        .transpose(0, 2, 1, 3)
    x8 = f8(xt)
    xres = np.empty((B, 128, KO, 2, SKP), F8)
    xres[:, :, :, 0, :] = f8(xt / 16.0)
    xres[:, :, :, 1, :] = f8(16.0 * (xt - x8.astype(np.float32)))

    def wtrip(W):
        Wp = 32.0 * np.ascontiguousarray(W, np.float32)
        Wr = Wp.reshape(KO, 128, H).transpose(1, 0, 2)  # [ki, ko, o]
        w8 = f8(Wr)
        dw8 = f8(16.0 * (Wr - w8.astype(np.float32)))
        w8s = f8(Wr / 16.0)
        return w8, dw8, w8s

    wq3 = wtrip(Wq)
    wk3 = wtrip(Wk)
    # packed QK triples: wqk[blk][ki, ko, iw, it, 128]
    wqk = np.empty((KO, 128, KO, 2, 3, 128), F8)
    for blk in range(KO):
        c = slice(blk * 128, blk * 128 + 128)
        for it in range(3):
            wqk[blk, :, :, 0, it, :] = wq3[it][:, :, c]
            wqk[blk, :, :, 1, it, :] = wk3[it][:, :, c]
    wv8, dwv8, swv8 = wtrip(Wv)
    wvres = np.stack([dwv8, swv8], axis=2)  # [128, KO, 2, H]
    wo16 = np.asarray(
        np.ascontiguousarray(Wo, np.float32).reshape(KO, 128, H)
        .transpose(1, 0, 2), BF16)
    common = {"wv8": wv8, "wvres": wvres, "wo": wo16,
              "ident": np.eye(128, dtype=BF16)}
    for b in range(KO):
        common[f"wqk{b}"] = wqk[b]
    return common, x8, xres


def kernel(hidden_states, Wq, bq, Wk, bk, Wv, bv, Wo, bo):
    from concourse.bass_utils import run_bass_kernel_spmd

    zero_bias = not (np.any(bq) or np.any(bk) or np.any(bv) or np.any(bo))
    key = ("nc", zero_bias)
    if key not in _CACHE:
        _CACHE[key] = _build_nc(zero_bias)
    nc = _CACHE[key]

    common, x8, xres = _prep(hidden_states, Wq, Wk, Wv, Wo)
    if not zero_bias:
        common.update({
            "bq32": np.ascontiguousarray(32.0 * bq, np.float32),
            "bk32": np.ascontiguousarray(32.0 * bk, np.float32),
            "bv32": np.asarray(32.0 * bv, BF16).reshape(1, H),
            "bo8": np.asarray(8.0 * bo, BF16).reshape(1, H),
            "ones": np.ones((1, 128), BF16),
        })
    in_maps = [dict(common, x8=x8[b], xres=xres[b]) for b in range(B)]

    res = run_bass_kernel_spmd(nc, in_maps, core_ids=list(range(B)))
    out = np.stack([np.asarray(r["out"]).astype(np.float32)
                    for r in res.results], axis=0)
    return out


if __name__ == "__main__":
    rng = np.random.default_rng(0)
    inputs = {
        "hidden_states": rng.standard_normal((B, S, H)).astype(np.float32),
        "Wq": (rng.standard_normal((H, H)) * 0.02).astype(np.float32),
        "bq": np.zeros(H, np.float32),
        "Wk": (rng.standard_normal((H, H)) * 0.02).astype(np.float32),
        "bk": np.zeros(H, np.float32),
        "Wv": (rng.standard_normal((H, H)) * 0.02).astype(np.float32),
        "bv": np.zeros(H, np.float32),
        "Wo": (rng.standard_normal((H, H)) * 0.02).astype(np.float32),
        "bo": np.zeros(H, np.float32),
    }
    got = kernel(**inputs)
    print("kernel output:", got.shape, got.dtype)
